# revision 28
# baseline (speedup 1.0000x reference)
"""Expert-parallel MoE MLP kernel for TRN2 (8 NeuronCores).

Reference computation (all experts, dense routing):
    hidden = einsum("bnd,edh->benh", x, w1); hidden = gelu(hidden)
    out    = einsum("benh,ehd->bnde", hidden, w2)        # [b, n, d4, e]

Sharding: expert-parallel, 2 experts per core (16 experts / 8 cores); x is
replicated. Each core computes, for its experts e:
    hT[e] = gelu(W1[e].T @ X.T)        # [h, tok] layout, h on partitions
    outT[e] = W2[e].T @ hT[e]          # [d4, tok] layout
which keeps the contraction dim on SBUF partitions for both matmuls with no
on-device transposes.

The whole data path is bf16 (PSUM accumulation stays f32): bf16 matmuls run
at the same 1 row/cycle as fp32r (216ns vs 227ns per 512-row matmul measured
- fast weight load hides the weight-load bubble fp32r pays), and bf16 halves
all DMA traffic including the output (upcast to f32 on the host; end-to-end
quantization error ~4e-3, well under the 2e-2 gate). fp8 DoubleRow was
measured at only ~2x fp32r MAC rate on this hardware (not the 4x the cost
model claims), which makes the 3-term hi/lo error-compensation scheme the
2e-2 gate requires a net loss - so bf16 it is. DMA is consolidated into few
dma_starts (each costs ~640ns of serialized sequencer config time) with the
first token tile's critical slices queued ahead of everything else. The
[e, d4, tok] device layout is re-interleaved to [b, n, d4, e] on the host.
"""

import sys

import numpy as np

for _p in ("/opt/trn_rl_repo", "/root/.axon_site/_ro/trn_rl_repo"):
    if _p not in sys.path:
        sys.path.append(_p)

import ml_dtypes

import concourse.bacc as bacc
import concourse.mybir as mybir
import concourse.tile as tile
from concourse.bass_utils import run_bass_kernel_spmd

F32 = mybir.dt.float32
BF16 = mybir.dt.bfloat16
NP_BF16 = ml_dtypes.bfloat16

N_CORES = 8
E = 16                 # total experts
E_LOC = E // N_CORES   # experts per core
D = 512                # model dim (contraction of mm1)
H = 512                # hidden dim (contraction of mm2)
D4 = 128               # output dim per expert
NTOK = 4 * 2048        # tokens
TT = 512               # token tile (matmul moving free dim)
P = 128


def _build_program():
    nc = bacc.Bacc("TRN2", target_bir_lowering=False, debug=False)
    xT = nc.declare_dram_parameter("xT", [D, NTOK], BF16, isOutput=False)
    w1 = nc.declare_dram_parameter("w1", [E_LOC, D, H], BF16, isOutput=False)
    w2 = nc.declare_dram_parameter("w2", [E_LOC, H, D4], BF16, isOutput=False)
    outT = nc.declare_dram_parameter("outT", [E_LOC, D4, NTOK], BF16, isOutput=True)

    gelu = mybir.ActivationFunctionType.Gelu
    n_dt = D // P   # 4 k-tiles of mm1
    n_ht = H // P   # 4 k-tiles of mm2

    with tile.TileContext(nc) as tc:
        with (
            tc.tile_pool(name="wpool", bufs=1) as wpool,
            tc.tile_pool(name="xpool", bufs=4) as xpool,
            tc.tile_pool(name="hpool", bufs=2) as hpool,
            tc.tile_pool(name="opool", bufs=4) as opool,
            tc.tile_pool(name="ps1p", bufs=4, space="PSUM") as ps1p,
            tc.tile_pool(name="ps2p", bufs=3, space="PSUM") as ps2p,
        ):
            # Weights resident in SBUF for the whole kernel, natural layout.
            w1_sb = wpool.tile([P, E_LOC, n_dt, H], BF16, name="w1_sb", tag="w1")
            w1_r = w1.rearrange("e (dt p) h -> p e dt h", p=P)
            w2_sb = wpool.tile([P, E_LOC, n_ht, D4], BF16, name="w2_sb", tag="w2")
            w2_r = w2.rearrange("e (ht p) d -> p e ht d", p=P)
            xT_r = xT.rearrange("(dt p) n -> p dt n", p=P)
            outT_r = outT.rearrange("e d n -> d e n")

            x_tiles = {}

            # Uniform token tiles. (Splitting the final tile to shorten the
            # drain was measured WORSE: the extra dma_start's completion
            # chain costs more than the smaller final transfer saves.)
            tiles = [(i * TT, TT) for i in range(NTOK // TT)]

            def load_x(i):
                t0, tl = tiles[i]
                x_sb = xpool.tile([P, n_dt, tl], BF16, name="x_sb", tag="x")
                nc.sync.dma_start(x_sb, xT_r[:, :, t0 : t0 + tl])
                x_tiles[i] = x_sb

            # Startup: the first matmul needs only x0[dt0] + w1[e0][dt0];
            # those two DMAs go first (each dma_start costs ~640ns of serial
            # sequencer config, so the critical ones must lead the queue).
            # All on the SP queue: issuing w1 from the Activation queue was
            # measured WORSE - the gelu table load serializes ahead of it.
            tok0 = slice(0, TT)
            x0_sb = xpool.tile([P, n_dt, TT], BF16, name="x_sb", tag="x")
            nc.sync.dma_start(x0_sb[:, 0], xT_r[:, 0, tok0])
            nc.sync.dma_start(w1_sb[:, 0, 0], w1_r[:, 0, 0])
            nc.sync.dma_start(x0_sb[:, 1], xT_r[:, 1, tok0])
            nc.sync.dma_start(w1_sb[:, 0, 1], w1_r[:, 0, 1])
            nc.sync.dma_start(x0_sb[:, 2:4], xT_r[:, 2:4, tok0])
            nc.sync.dma_start(w1_sb[:, 0, 2:4], w1_r[:, 0, 2:4])
            x_tiles[0] = x0_sb
            # w1[e1] before w2: mm1(e1) needs it ~2us before mm2(e0) needs w2.
            for e in range(1, E_LOC):
                nc.sync.dma_start(w1_sb[:, e], w1_r[:, e])
            # Prefetch x1/x2 ahead of w2: tile 1 starts while the clock is
            # still ramping, and its x otherwise lands marginally late
            # (recurring ~0.5us stream gap at the tile 0 -> 1 boundary);
            # w2 is not needed until mm2(e0) ~8us later.
            load_x(1)
            load_x(2)
            for e in range(E_LOC):
                nc.sync.dma_start(w2_sb[:, e], w2_r[:, e])

            for i, (t0, tl) in enumerate(tiles):
                if i not in x_tiles:
                    load_x(i)
                x_sb = x_tiles.pop(i)
                hT_tiles = []
                for e in range(E_LOC):
                    hT_sb = hpool.tile([P, n_ht, tl], BF16, name="hT_sb", tag="h")
                    for ht in range(n_ht):
                        ps1 = ps1p.tile([P, TT], F32, name="ps1", tag="ps1")
                        for dt_i in range(n_dt):
                            nc.tensor.matmul(
                                ps1[:, :tl],
                                w1_sb[:, e, dt_i, ht * P : (ht + 1) * P],
                                x_sb[:, dt_i],
                                start=(dt_i == 0),
                                stop=(dt_i == n_dt - 1),
                            )
                        nc.scalar.activation(hT_sb[:, ht, :], ps1[:, :tl], gelu)
                    hT_tiles.append(hT_sb)
                # Both experts' outputs stage into one tile and ship as ONE
                # dma_start: each DMA carries a ~0.7us completion chain, so
                # halving the count shortens the end-of-kernel drain.
                o_sb = opool.tile([P, E_LOC, tl], BF16, name="o_sb", tag="o")
                for e in range(E_LOC):
                    ps2 = ps2p.tile([P, TT], F32, name="ps2", tag="ps2")
                    for ht in range(n_ht):
                        nc.tensor.matmul(
                            ps2[:, :tl],
                            w2_sb[:, e, ht, :],
                            hT_tiles[e][:, ht, :],
                            start=(ht == 0),
                            stop=(ht == n_ht - 1),
                        )
                    nc.vector.tensor_copy(o_sb[:, e], ps2[:, :tl])
                nc.sync.dma_start(outT_r[:, :, t0 : t0 + tl], o_sb)

    nc.finalize()
    return nc


_NC = None


def _get_program():
    global _NC
    if _NC is None:
        _NC = _build_program()
    return _NC


def _prep_in_maps(x, w1, w2):
    """Host-side bf16 cast + transpose; returns per-core input maps."""
    X = np.ascontiguousarray(x.reshape(NTOK, D)).astype(np.float32, copy=False)
    xT = np.ascontiguousarray(X.T.astype(NP_BF16))

    in_maps = []
    for c in range(N_CORES):
        w1c = np.ascontiguousarray(
            w1[c * E_LOC : (c + 1) * E_LOC].astype(NP_BF16)
        )
        w2c = np.ascontiguousarray(
            w2[c * E_LOC : (c + 1) * E_LOC].astype(NP_BF16)
        )
        in_maps.append({"xT": xT, "w1": w1c, "w2": w2c})
    return in_maps


def kernel(x: np.ndarray, w1: np.ndarray, w2: np.ndarray, **_) -> np.ndarray:
    """Full inputs in, full output out; expert-parallel across 8 NeuronCores."""
    nc = _get_program()
    in_maps = _prep_in_maps(x, w1, w2)
    res = run_bass_kernel_spmd(nc, in_maps, list(range(N_CORES)))

    full = np.stack(
        [res.results[c]["outT"].astype(np.float32) for c in range(N_CORES)], axis=0
    )
    full = full.reshape(E, D4, NTOK)              # [e, d4, tok]
    out = full.transpose(2, 1, 0)                 # [tok, d4, e]
    return np.ascontiguousarray(out.reshape(4, 2048, D4, E), dtype=np.float32)


# revision 29
# speedup vs baseline: 1.0020x; 1.0020x over previous
"""Expert-parallel MoE MLP kernel for TRN2 (8 NeuronCores).

Reference computation (all experts, dense routing):
    hidden = einsum("bnd,edh->benh", x, w1); hidden = gelu(hidden)
    out    = einsum("benh,ehd->bnde", hidden, w2)        # [b, n, d4, e]

Sharding: expert-parallel, 2 experts per core (16 experts / 8 cores); x is
replicated. Each core computes, for its experts e:
    hT[e] = gelu(W1[e].T @ X.T)        # [h, tok] layout, h on partitions
    outT[e] = W2[e].T @ hT[e]          # [d4, tok] layout
which keeps the contraction dim on SBUF partitions for both matmuls with no
on-device transposes.

The whole data path is bf16 (PSUM accumulation stays f32): bf16 matmuls run
at the same 1 row/cycle as fp32r (216ns vs 227ns per 512-row matmul measured
- fast weight load hides the weight-load bubble fp32r pays), and bf16 halves
all DMA traffic including the output (upcast to f32 on the host; end-to-end
quantization error ~4e-3, well under the 2e-2 gate). fp8 DoubleRow was
measured at only ~2x fp32r MAC rate on this hardware (not the 4x the cost
model claims), which makes the 3-term hi/lo error-compensation scheme the
2e-2 gate requires a net loss - so bf16 it is. DMA is consolidated into few
dma_starts (each costs ~640ns of serialized sequencer config time) with the
first token tile's critical slices queued ahead of everything else. The
[e, d4, tok] device layout is re-interleaved to [b, n, d4, e] on the host.
"""

import sys

import numpy as np

for _p in ("/opt/trn_rl_repo", "/root/.axon_site/_ro/trn_rl_repo"):
    if _p not in sys.path:
        sys.path.append(_p)

import ml_dtypes

import concourse.bacc as bacc
import concourse.mybir as mybir
import concourse.tile as tile
from concourse.bass_utils import run_bass_kernel_spmd

F32 = mybir.dt.float32
BF16 = mybir.dt.bfloat16
NP_BF16 = ml_dtypes.bfloat16

N_CORES = 8
E = 16                 # total experts
E_LOC = E // N_CORES   # experts per core
D = 512                # model dim (contraction of mm1)
H = 512                # hidden dim (contraction of mm2)
D4 = 128               # output dim per expert
NTOK = 4 * 2048        # tokens
TT = 512               # token tile (matmul moving free dim)
P = 128


def _build_program():
    nc = bacc.Bacc("TRN2", target_bir_lowering=False, debug=False)
    xT = nc.declare_dram_parameter("xT", [D, NTOK], BF16, isOutput=False)
    w1 = nc.declare_dram_parameter("w1", [E_LOC, D, H], BF16, isOutput=False)
    w2 = nc.declare_dram_parameter("w2", [E_LOC, H, D4], BF16, isOutput=False)
    outT = nc.declare_dram_parameter("outT", [E_LOC, D4, NTOK], BF16, isOutput=True)

    gelu = mybir.ActivationFunctionType.Gelu
    n_dt = D // P   # 4 k-tiles of mm1
    n_ht = H // P   # 4 k-tiles of mm2

    with tile.TileContext(nc) as tc:
        with (
            tc.tile_pool(name="wpool", bufs=1) as wpool,
            tc.tile_pool(name="xpool", bufs=4) as xpool,
            tc.tile_pool(name="hpool", bufs=2) as hpool,
            tc.tile_pool(name="opool", bufs=4) as opool,
            tc.tile_pool(name="ps1p", bufs=4, space="PSUM") as ps1p,
            tc.tile_pool(name="ps2p", bufs=3, space="PSUM") as ps2p,
        ):
            # Weights resident in SBUF for the whole kernel, natural layout.
            w1_sb = wpool.tile([P, E_LOC, n_dt, H], BF16, name="w1_sb", tag="w1")
            w1_r = w1.rearrange("e (dt p) h -> p e dt h", p=P)
            w2_sb = wpool.tile([P, E_LOC, n_ht, D4], BF16, name="w2_sb", tag="w2")
            w2_r = w2.rearrange("e (ht p) d -> p e ht d", p=P)
            xT_r = xT.rearrange("(dt p) n -> p dt n", p=P)
            outT_r = outT.rearrange("e d n -> d e n")

            x_tiles = {}

            # Uniform token tiles. (Splitting the final tile to shorten the
            # drain was measured WORSE: the extra dma_start's completion
            # chain costs more than the smaller final transfer saves.)
            tiles = [(i * TT, TT) for i in range(NTOK // TT)]

            def load_x(i):
                t0, tl = tiles[i]
                x_sb = xpool.tile([P, n_dt, tl], BF16, name="x_sb", tag="x")
                nc.sync.dma_start(x_sb, xT_r[:, :, t0 : t0 + tl])
                x_tiles[i] = x_sb

            # Startup: the first matmul needs only x0[dt0] + w1[e0][dt0];
            # those two DMAs go first (each dma_start costs ~640ns of serial
            # sequencer config, so the critical ones must lead the queue).
            # All on the SP queue: issuing w1 from the Activation queue was
            # measured WORSE - the gelu table load serializes ahead of it.
            tok0 = slice(0, TT)
            x0_sb = xpool.tile([P, n_dt, TT], BF16, name="x_sb", tag="x")
            nc.sync.dma_start(x0_sb[:, 0], xT_r[:, 0, tok0])
            nc.sync.dma_start(w1_sb[:, 0, 0], w1_r[:, 0, 0])
            nc.sync.dma_start(x0_sb[:, 1], xT_r[:, 1, tok0])
            nc.sync.dma_start(w1_sb[:, 0, 1], w1_r[:, 0, 1])
            nc.sync.dma_start(x0_sb[:, 2:4], xT_r[:, 2:4, tok0])
            nc.sync.dma_start(w1_sb[:, 0, 2:4], w1_r[:, 0, 2:4])
            x_tiles[0] = x0_sb
            # w1[e1] before w2: mm1(e1) needs it ~2us before mm2(e0) needs w2.
            for e in range(1, E_LOC):
                nc.sync.dma_start(w1_sb[:, e], w1_r[:, e])
            # Prefetch x1/x2 ahead of w2: tile 1 starts while the clock is
            # still ramping, and its x otherwise lands marginally late
            # (recurring ~0.5us stream gap at the tile 0 -> 1 boundary);
            # w2 is not needed until mm2(e0) ~8us later.
            load_x(1)
            load_x(2)
            for e in range(E_LOC):
                nc.sync.dma_start(w2_sb[:, e], w2_r[:, e])

            for i, (t0, tl) in enumerate(tiles):
                if i not in x_tiles:
                    load_x(i)
                x_sb = x_tiles.pop(i)
                hT_tiles = []
                for e in range(E_LOC):
                    hT_sb = hpool.tile([P, n_ht, tl], BF16, name="hT_sb", tag="h")
                    for ht in range(n_ht):
                        ps1 = ps1p.tile([P, TT], F32, name="ps1", tag="ps1")
                        for dt_i in range(n_dt):
                            nc.tensor.matmul(
                                ps1[:, :tl],
                                w1_sb[:, e, dt_i, ht * P : (ht + 1) * P],
                                x_sb[:, dt_i],
                                start=(dt_i == 0),
                                stop=(dt_i == n_dt - 1),
                            )
                        nc.scalar.activation(hT_sb[:, ht, :], ps1[:, :tl], gelu)
                    hT_tiles.append(hT_sb)
                # Both experts' outputs stage into one tile and ship as ONE
                # dma_start: each DMA carries a ~0.7us completion chain, so
                # halving the count shortens the end-of-kernel drain.
                o_sb = opool.tile([P, E_LOC, tl], BF16, name="o_sb", tag="o")
                for e in range(E_LOC):
                    ps2 = ps2p.tile([P, TT], F32, name="ps2", tag="ps2")
                    for ht in range(n_ht):
                        nc.tensor.matmul(
                            ps2[:, :tl],
                            w2_sb[:, e, ht, :],
                            hT_tiles[e][:, ht, :],
                            start=(ht == 0),
                            stop=(ht == n_ht - 1),
                        )
                    if i == len(tiles) - 1 and e == E_LOC - 1:
                        # Final drain: the very last PSUM->SBUF copy is on
                        # the critical path; split it across DVE and Scalar
                        # so the halves run in parallel (~0.35us vs 0.69us)
                        # before the single merged DMA fires.
                        hv = tl // 2
                        nc.vector.tensor_copy(o_sb[:, e, :hv], ps2[:, :hv])
                        nc.scalar.copy(o_sb[:, e, hv:], ps2[:, hv:tl])
                    else:
                        nc.vector.tensor_copy(o_sb[:, e], ps2[:, :tl])
                nc.sync.dma_start(outT_r[:, :, t0 : t0 + tl], o_sb)

    nc.finalize()
    return nc


_NC = None


def _get_program():
    global _NC
    if _NC is None:
        _NC = _build_program()
    return _NC


def _prep_in_maps(x, w1, w2):
    """Host-side bf16 cast + transpose; returns per-core input maps."""
    X = np.ascontiguousarray(x.reshape(NTOK, D)).astype(np.float32, copy=False)
    xT = np.ascontiguousarray(X.T.astype(NP_BF16))

    in_maps = []
    for c in range(N_CORES):
        w1c = np.ascontiguousarray(
            w1[c * E_LOC : (c + 1) * E_LOC].astype(NP_BF16)
        )
        w2c = np.ascontiguousarray(
            w2[c * E_LOC : (c + 1) * E_LOC].astype(NP_BF16)
        )
        in_maps.append({"xT": xT, "w1": w1c, "w2": w2c})
    return in_maps


def kernel(x: np.ndarray, w1: np.ndarray, w2: np.ndarray, **_) -> np.ndarray:
    """Full inputs in, full output out; expert-parallel across 8 NeuronCores."""
    nc = _get_program()
    in_maps = _prep_in_maps(x, w1, w2)
    res = run_bass_kernel_spmd(nc, in_maps, list(range(N_CORES)))

    full = np.stack(
        [res.results[c]["outT"].astype(np.float32) for c in range(N_CORES)], axis=0
    )
    full = full.reshape(E, D4, NTOK)              # [e, d4, tok]
    out = full.transpose(2, 1, 0)                 # [tok, d4, e]
    return np.ascontiguousarray(out.reshape(4, 2048, D4, E), dtype=np.float32)


# revision 31
# speedup vs baseline: 1.0114x; 1.0094x over previous
"""Expert-parallel MoE MLP kernel for TRN2 (8 NeuronCores).

Reference computation (all experts, dense routing):
    hidden = einsum("bnd,edh->benh", x, w1); hidden = gelu(hidden)
    out    = einsum("benh,ehd->bnde", hidden, w2)        # [b, n, d4, e]

Sharding: expert-parallel, 2 experts per core (16 experts / 8 cores); x is
replicated. Each core computes, for its experts e:
    hT[e] = gelu(W1[e].T @ X.T)        # [h, tok] layout, h on partitions
    outT[e] = W2[e].T @ hT[e]          # [d4, tok] layout
which keeps the contraction dim on SBUF partitions for both matmuls with no
on-device transposes.

The whole data path is bf16 (PSUM accumulation stays f32): bf16 matmuls run
at the same 1 row/cycle as fp32r (216ns vs 227ns per 512-row matmul measured
- fast weight load hides the weight-load bubble fp32r pays), and bf16 halves
all DMA traffic including the output (upcast to f32 on the host; end-to-end
quantization error ~4e-3, well under the 2e-2 gate). fp8 DoubleRow was
measured at only ~2x fp32r MAC rate on this hardware (not the 4x the cost
model claims), which makes the 3-term hi/lo error-compensation scheme the
2e-2 gate requires a net loss - so bf16 it is. DMA is consolidated into few
dma_starts (each costs ~640ns of serialized sequencer config time) with the
first token tile's critical slices queued ahead of everything else. The
[e, d4, tok] device layout is re-interleaved to [b, n, d4, e] on the host.
"""

import sys

import numpy as np

for _p in ("/opt/trn_rl_repo", "/root/.axon_site/_ro/trn_rl_repo"):
    if _p not in sys.path:
        sys.path.append(_p)

import ml_dtypes

import concourse.bacc as bacc
import concourse.mybir as mybir
import concourse.tile as tile
from concourse.bass_utils import run_bass_kernel_spmd

F32 = mybir.dt.float32
BF16 = mybir.dt.bfloat16
NP_BF16 = ml_dtypes.bfloat16

N_CORES = 8
E = 16                 # total experts
E_LOC = E // N_CORES   # experts per core
D = 512                # model dim (contraction of mm1)
H = 512                # hidden dim (contraction of mm2)
D4 = 128               # output dim per expert
NTOK = 4 * 2048        # tokens
TT = 512               # token tile (matmul moving free dim)
P = 128


def _build_program():
    nc = bacc.Bacc("TRN2", target_bir_lowering=False, debug=False)
    xT = nc.declare_dram_parameter("xT", [D, NTOK], BF16, isOutput=False)
    w1 = nc.declare_dram_parameter("w1", [E_LOC, D, H], BF16, isOutput=False)
    w2 = nc.declare_dram_parameter("w2", [E_LOC, H, D4], BF16, isOutput=False)
    outT = nc.declare_dram_parameter("outT", [E_LOC, D4, NTOK], BF16, isOutput=True)

    gelu = mybir.ActivationFunctionType.Gelu
    n_dt = D // P   # 4 k-tiles of mm1
    n_ht = H // P   # 4 k-tiles of mm2

    with tile.TileContext(nc) as tc:
        with (
            tc.tile_pool(name="wpool", bufs=1) as wpool,
            tc.tile_pool(name="xpool", bufs=4) as xpool,
            tc.tile_pool(name="hpool", bufs=2) as hpool,
            tc.tile_pool(name="opool", bufs=4) as opool,
            tc.tile_pool(name="ps1p", bufs=4, space="PSUM") as ps1p,
            tc.tile_pool(name="ps2p", bufs=3, space="PSUM") as ps2p,
        ):
            # Weights resident in SBUF for the whole kernel, natural layout.
            w1_sb = wpool.tile([P, E_LOC, n_dt, H], BF16, name="w1_sb", tag="w1")
            w1_r = w1.rearrange("e (dt p) h -> p e dt h", p=P)
            w2_sb = wpool.tile([P, E_LOC, n_ht, D4], BF16, name="w2_sb", tag="w2")
            w2_r = w2.rearrange("e (ht p) d -> p e ht d", p=P)
            xT_r = xT.rearrange("(dt p) n -> p dt n", p=P)
            outT_r = outT.rearrange("e d n -> d e n")

            x_tiles = {}

            # Uniform token tiles. (Splitting the final tile to shorten the
            # drain was measured WORSE: the extra dma_start's completion
            # chain costs more than the smaller final transfer saves.)
            tiles = [(i * TT, TT) for i in range(NTOK // TT)]

            def load_x(i):
                t0, tl = tiles[i]
                x_sb = xpool.tile([P, n_dt, tl], BF16, name="x_sb", tag="x")
                nc.sync.dma_start(x_sb, xT_r[:, :, t0 : t0 + tl])
                x_tiles[i] = x_sb

            # Startup: the first matmul needs only x0[dt0] + w1[e0][dt0];
            # those two DMAs go first (each dma_start costs ~640ns of serial
            # sequencer config, so the critical ones must lead the queue).
            # All on the SP queue: issuing w1 from the Activation queue was
            # measured WORSE - the gelu table load serializes ahead of it.
            tok0 = slice(0, TT)
            x0_sb = xpool.tile([P, n_dt, TT], BF16, name="x_sb", tag="x")
            nc.sync.dma_start(x0_sb[:, 0], xT_r[:, 0, tok0])
            nc.sync.dma_start(w1_sb[:, 0, 0], w1_r[:, 0, 0])
            nc.sync.dma_start(x0_sb[:, 1], xT_r[:, 1, tok0])
            nc.sync.dma_start(w1_sb[:, 0, 1], w1_r[:, 0, 1])
            nc.sync.dma_start(x0_sb[:, 2:4], xT_r[:, 2:4, tok0])
            nc.sync.dma_start(w1_sb[:, 0, 2:4], w1_r[:, 0, 2:4])
            x_tiles[0] = x0_sb
            # w1[e1] before w2: mm1(e1) needs it ~2us before mm2(e0) needs w2.
            for e in range(1, E_LOC):
                nc.sync.dma_start(w1_sb[:, e], w1_r[:, e])
            # Prefetch x1/x2 ahead of w2: tile 1 starts while the clock is
            # still ramping, and its x otherwise lands marginally late
            # (recurring ~0.5us stream gap at the tile 0 -> 1 boundary);
            # w2 is not needed until mm2(e0) ~8us later.
            load_x(1)
            load_x(2)
            for e in range(E_LOC):
                nc.sync.dma_start(w2_sb[:, e], w2_r[:, e])

            for i, (t0, tl) in enumerate(tiles):
                if i not in x_tiles:
                    load_x(i)
                x_sb = x_tiles.pop(i)
                hT_tiles = []
                for e in range(E_LOC):
                    hT_sb = hpool.tile([P, n_ht, tl], BF16, name="hT_sb", tag="h")
                    for ht in range(n_ht):
                        ps1 = ps1p.tile([P, TT], F32, name="ps1", tag="ps1")
                        for dt_i in range(n_dt):
                            nc.tensor.matmul(
                                ps1[:, :tl],
                                w1_sb[:, e, dt_i, ht * P : (ht + 1) * P],
                                x_sb[:, dt_i],
                                start=(dt_i == 0),
                                stop=(dt_i == n_dt - 1),
                            )
                        nc.scalar.activation(hT_sb[:, ht, :], ps1[:, :tl], gelu)
                    hT_tiles.append(hT_sb)
                # Both experts' outputs stage into one tile and ship as ONE
                # dma_start: each DMA carries a ~0.7us completion chain, so
                # halving the count shortens the end-of-kernel drain.
                o_sb = opool.tile([P, E_LOC, tl], BF16, name="o_sb", tag="o")
                for e in range(E_LOC):
                    ps2 = ps2p.tile([P, TT], F32, name="ps2", tag="ps2")
                    if i == len(tiles) - 1 and e == E_LOC - 1:
                        # Final drain: run the last expert's mm2 as two
                        # token-half accumulation groups so the first half's
                        # copy overlaps the second half's matmuls; only a
                        # half-copy (~0.43us, PSUM-access-latency bound)
                        # remains serial after the last matmul. All on DVE -
                        # Scalar's sem propagation (~517ns vs DVE's ~40ns)
                        # serializes any cross-engine variant.
                        for hv0, hv1 in ((0, tl // 2), (tl // 2, tl)):
                            for ht in range(n_ht):
                                nc.tensor.matmul(
                                    ps2[:, hv0:hv1],
                                    w2_sb[:, e, ht, :],
                                    hT_tiles[e][:, ht, hv0:hv1],
                                    start=(ht == 0),
                                    stop=(ht == n_ht - 1),
                                )
                            nc.vector.tensor_copy(
                                o_sb[:, e, hv0:hv1], ps2[:, hv0:hv1]
                            )
                    else:
                        for ht in range(n_ht):
                            nc.tensor.matmul(
                                ps2[:, :tl],
                                w2_sb[:, e, ht, :],
                                hT_tiles[e][:, ht, :],
                                start=(ht == 0),
                                stop=(ht == n_ht - 1),
                            )
                        nc.vector.tensor_copy(o_sb[:, e], ps2[:, :tl])
                nc.sync.dma_start(outT_r[:, :, t0 : t0 + tl], o_sb)

    nc.finalize()
    return nc


_NC = None


def _get_program():
    global _NC
    if _NC is None:
        _NC = _build_program()
    return _NC


def _prep_in_maps(x, w1, w2):
    """Host-side bf16 cast + transpose; returns per-core input maps."""
    X = np.ascontiguousarray(x.reshape(NTOK, D)).astype(np.float32, copy=False)
    xT = np.ascontiguousarray(X.T.astype(NP_BF16))

    in_maps = []
    for c in range(N_CORES):
        w1c = np.ascontiguousarray(
            w1[c * E_LOC : (c + 1) * E_LOC].astype(NP_BF16)
        )
        w2c = np.ascontiguousarray(
            w2[c * E_LOC : (c + 1) * E_LOC].astype(NP_BF16)
        )
        in_maps.append({"xT": xT, "w1": w1c, "w2": w2c})
    return in_maps


def kernel(x: np.ndarray, w1: np.ndarray, w2: np.ndarray, **_) -> np.ndarray:
    """Full inputs in, full output out; expert-parallel across 8 NeuronCores."""
    nc = _get_program()
    in_maps = _prep_in_maps(x, w1, w2)
    res = run_bass_kernel_spmd(nc, in_maps, list(range(N_CORES)))

    full = np.stack(
        [res.results[c]["outT"].astype(np.float32) for c in range(N_CORES)], axis=0
    )
    full = full.reshape(E, D4, NTOK)              # [e, d4, tok]
    out = full.transpose(2, 1, 0)                 # [tok, d4, e]
    return np.ascontiguousarray(out.reshape(4, 2048, D4, E), dtype=np.float32)


# revision 32
# speedup vs baseline: 1.0120x; 1.0005x over previous
"""Expert-parallel MoE MLP kernel for TRN2 (8 NeuronCores).

Reference computation (all experts, dense routing):
    hidden = einsum("bnd,edh->benh", x, w1); hidden = gelu(hidden)
    out    = einsum("benh,ehd->bnde", hidden, w2)        # [b, n, d4, e]

Sharding: expert-parallel, 2 experts per core (16 experts / 8 cores); x is
replicated. Each core computes, for its experts e:
    hT[e] = gelu(W1[e].T @ X.T)        # [h, tok] layout, h on partitions
    outT[e] = W2[e].T @ hT[e]          # [d4, tok] layout
which keeps the contraction dim on SBUF partitions for both matmuls with no
on-device transposes.

The whole data path is bf16 (PSUM accumulation stays f32): bf16 matmuls run
at the same 1 row/cycle as fp32r (216ns vs 227ns per 512-row matmul measured
- fast weight load hides the weight-load bubble fp32r pays), and bf16 halves
all DMA traffic including the output (upcast to f32 on the host; end-to-end
quantization error ~4e-3, well under the 2e-2 gate). fp8 DoubleRow was
measured at only ~2x fp32r MAC rate on this hardware (not the 4x the cost
model claims), which makes the 3-term hi/lo error-compensation scheme the
2e-2 gate requires a net loss - so bf16 it is. DMA is consolidated into few
dma_starts (each costs ~640ns of serialized sequencer config time) with the
first token tile's critical slices queued ahead of everything else. The
[e, d4, tok] device layout is re-interleaved to [b, n, d4, e] on the host.
"""

import sys

import numpy as np

for _p in ("/opt/trn_rl_repo", "/root/.axon_site/_ro/trn_rl_repo"):
    if _p not in sys.path:
        sys.path.append(_p)

import ml_dtypes

import concourse.bacc as bacc
import concourse.mybir as mybir
import concourse.tile as tile
from concourse.bass_utils import run_bass_kernel_spmd

F32 = mybir.dt.float32
BF16 = mybir.dt.bfloat16
NP_BF16 = ml_dtypes.bfloat16

N_CORES = 8
E = 16                 # total experts
E_LOC = E // N_CORES   # experts per core
D = 512                # model dim (contraction of mm1)
H = 512                # hidden dim (contraction of mm2)
D4 = 128               # output dim per expert
NTOK = 4 * 2048        # tokens
TT = 512               # token tile (matmul moving free dim)
P = 128


def _build_program():
    nc = bacc.Bacc("TRN2", target_bir_lowering=False, debug=False)
    xT = nc.declare_dram_parameter("xT", [D, NTOK], BF16, isOutput=False)
    w1 = nc.declare_dram_parameter("w1", [E_LOC, D, H], BF16, isOutput=False)
    w2 = nc.declare_dram_parameter("w2", [E_LOC, H, D4], BF16, isOutput=False)
    outT = nc.declare_dram_parameter("outT", [E_LOC, D4, NTOK], BF16, isOutput=True)

    gelu = mybir.ActivationFunctionType.Gelu
    n_dt = D // P   # 4 k-tiles of mm1
    n_ht = H // P   # 4 k-tiles of mm2

    with tile.TileContext(nc) as tc:
        with (
            tc.tile_pool(name="wpool", bufs=1) as wpool,
            tc.tile_pool(name="xpool", bufs=4) as xpool,
            tc.tile_pool(name="hpool", bufs=2) as hpool,
            tc.tile_pool(name="opool", bufs=4) as opool,
            tc.tile_pool(name="ps1p", bufs=4, space="PSUM") as ps1p,
            tc.tile_pool(name="ps2p", bufs=3, space="PSUM") as ps2p,
        ):
            # Weights resident in SBUF for the whole kernel, natural layout.
            w1_sb = wpool.tile([P, E_LOC, n_dt, H], BF16, name="w1_sb", tag="w1")
            w1_r = w1.rearrange("e (dt p) h -> p e dt h", p=P)
            w2_sb = wpool.tile([P, E_LOC, n_ht, D4], BF16, name="w2_sb", tag="w2")
            w2_r = w2.rearrange("e (ht p) d -> p e ht d", p=P)
            xT_r = xT.rearrange("(dt p) n -> p dt n", p=P)
            outT_r = outT.rearrange("e d n -> d e n")

            x_tiles = {}

            # Uniform token tiles. (Splitting the final tile to shorten the
            # drain was measured WORSE: the extra dma_start's completion
            # chain costs more than the smaller final transfer saves.)
            tiles = [(i * TT, TT) for i in range(NTOK // TT)]

            def load_x(i):
                t0, tl = tiles[i]
                x_sb = xpool.tile([P, n_dt, tl], BF16, name="x_sb", tag="x")
                nc.sync.dma_start(x_sb, xT_r[:, :, t0 : t0 + tl])
                x_tiles[i] = x_sb

            # Startup: the first matmul needs only x0[dt0] + w1[e0][dt0];
            # those two DMAs go first (each dma_start costs ~640ns of serial
            # sequencer config, so the critical ones must lead the queue).
            # All on the SP queue: issuing w1 from the Activation queue was
            # measured WORSE - the gelu table load serializes ahead of it.
            tok0 = slice(0, TT)
            x0_sb = xpool.tile([P, n_dt, TT], BF16, name="x_sb", tag="x")
            nc.sync.dma_start(x0_sb[:, 0], xT_r[:, 0, tok0])
            nc.sync.dma_start(w1_sb[:, 0, 0], w1_r[:, 0, 0])
            nc.sync.dma_start(x0_sb[:, 1], xT_r[:, 1, tok0])
            nc.sync.dma_start(w1_sb[:, 0, 1], w1_r[:, 0, 1])
            nc.sync.dma_start(x0_sb[:, 2:4], xT_r[:, 2:4, tok0])
            nc.sync.dma_start(w1_sb[:, 0, 2:4], w1_r[:, 0, 2:4])
            x_tiles[0] = x0_sb
            # w1[e1] before w2: mm1(e1) needs it ~2us before mm2(e0) needs w2.
            for e in range(1, E_LOC):
                nc.sync.dma_start(w1_sb[:, e], w1_r[:, e])
            # Prefetch x1/x2 ahead of w2: tile 1 starts while the clock is
            # still ramping, and its x otherwise lands marginally late
            # (recurring ~0.5us stream gap at the tile 0 -> 1 boundary);
            # w2 is not needed until mm2(e0) ~8us later.
            load_x(1)
            load_x(2)
            for e in range(E_LOC):
                nc.sync.dma_start(w2_sb[:, e], w2_r[:, e])

            for i, (t0, tl) in enumerate(tiles):
                if i not in x_tiles:
                    load_x(i)
                x_sb = x_tiles.pop(i)
                hT_tiles = []
                for e in range(E_LOC):
                    hT_sb = hpool.tile([P, n_ht, tl], BF16, name="hT_sb", tag="h")
                    for ht in range(n_ht):
                        ps1 = ps1p.tile([P, TT], F32, name="ps1", tag="ps1")
                        for dt_i in range(n_dt):
                            nc.tensor.matmul(
                                ps1[:, :tl],
                                w1_sb[:, e, dt_i, ht * P : (ht + 1) * P],
                                x_sb[:, dt_i],
                                start=(dt_i == 0),
                                stop=(dt_i == n_dt - 1),
                            )
                        nc.scalar.activation(hT_sb[:, ht, :], ps1[:, :tl], gelu)
                    hT_tiles.append(hT_sb)
                # Both experts' outputs stage into one tile and ship as ONE
                # dma_start: each DMA carries a ~0.7us completion chain, so
                # halving the count shortens the end-of-kernel drain.
                o_sb = opool.tile([P, E_LOC, tl], BF16, name="o_sb", tag="o")
                for e in range(E_LOC):
                    if i == len(tiles) - 1 and e == E_LOC - 1:
                        # Final drain: run the last expert's mm2 as two
                        # token-half accumulation groups in SEPARATE psum
                        # tiles (separate banks - sharing one tile adds a
                        # false copy-vs-zeroing dependency that stalls the
                        # second group ~0.7us) so the first half's copy
                        # overlaps the second half's matmuls; only a
                        # half-copy (~0.43us, PSUM-access-latency bound)
                        # remains serial after the last matmul. All on DVE -
                        # Scalar's sem propagation (~517ns vs DVE's ~40ns)
                        # serializes any cross-engine variant.
                        for hv0, hv1 in ((0, tl // 2), (tl // 2, tl)):
                            psh = ps2p.tile(
                                [P, TT // 2], F32, name="ps2", tag="ps2"
                            )
                            for ht in range(n_ht):
                                nc.tensor.matmul(
                                    psh,
                                    w2_sb[:, e, ht, :],
                                    hT_tiles[e][:, ht, hv0:hv1],
                                    start=(ht == 0),
                                    stop=(ht == n_ht - 1),
                                )
                            nc.vector.tensor_copy(o_sb[:, e, hv0:hv1], psh)
                    else:
                        ps2 = ps2p.tile([P, TT], F32, name="ps2", tag="ps2")
                        for ht in range(n_ht):
                            nc.tensor.matmul(
                                ps2[:, :tl],
                                w2_sb[:, e, ht, :],
                                hT_tiles[e][:, ht, :],
                                start=(ht == 0),
                                stop=(ht == n_ht - 1),
                            )
                        nc.vector.tensor_copy(o_sb[:, e], ps2[:, :tl])
                nc.sync.dma_start(outT_r[:, :, t0 : t0 + tl], o_sb)

    nc.finalize()
    return nc


_NC = None


def _get_program():
    global _NC
    if _NC is None:
        _NC = _build_program()
    return _NC


def _prep_in_maps(x, w1, w2):
    """Host-side bf16 cast + transpose; returns per-core input maps."""
    X = np.ascontiguousarray(x.reshape(NTOK, D)).astype(np.float32, copy=False)
    xT = np.ascontiguousarray(X.T.astype(NP_BF16))

    in_maps = []
    for c in range(N_CORES):
        w1c = np.ascontiguousarray(
            w1[c * E_LOC : (c + 1) * E_LOC].astype(NP_BF16)
        )
        w2c = np.ascontiguousarray(
            w2[c * E_LOC : (c + 1) * E_LOC].astype(NP_BF16)
        )
        in_maps.append({"xT": xT, "w1": w1c, "w2": w2c})
    return in_maps


def kernel(x: np.ndarray, w1: np.ndarray, w2: np.ndarray, **_) -> np.ndarray:
    """Full inputs in, full output out; expert-parallel across 8 NeuronCores."""
    nc = _get_program()
    in_maps = _prep_in_maps(x, w1, w2)
    res = run_bass_kernel_spmd(nc, in_maps, list(range(N_CORES)))

    full = np.stack(
        [res.results[c]["outT"].astype(np.float32) for c in range(N_CORES)], axis=0
    )
    full = full.reshape(E, D4, NTOK)              # [e, d4, tok]
    out = full.transpose(2, 1, 0)                 # [tok, d4, e]
    return np.ascontiguousarray(out.reshape(4, 2048, D4, E), dtype=np.float32)
